# revision 31
# baseline (speedup 1.0000x reference)
"""AttentionPooling (segment softmax-pool) Trainium2 kernel.

Math (per reference):
    h      = gelu(x @ W1 + b1)            # [N, H]
    s      = h @ W2 + b2                  # [N]
    w      = softmax_per_segment(s)       # segments from sorted `batch`
    pooled = segment_sum(w[:, None] * x)  # [B, D]

Strategy (8 NeuronCores, data-parallel over N):
  - Shard rows across 8 cores. Each core streams its rows once (bf16, both
    natural and DMA-transposed layouts), in groups of KST macro-tiles
    (one macro = 512 rows), computing:
      * scores via the tiny MLP on the tensor engine (bf16 in, f32 psum)
      * e = exp(s + b2) once per group (avoids ACT table thrash)
      * a one-hot-times-e matrix A[row, seg-in-window] built with
        iota/is_equal on the vector engine (window = [b_lo_m, b_lo_m + W))
      * windowed pooled partials P_m[W, D] = sum_rows e_i * x_i via matmul
        (A stationary, x moving), f32 PSUM accumulation
  - Device ships per-macro windows P_m and per-row e back to HBM.
  - Host scatter-adds the windows at their (host-known) b_lo_m offsets,
    computes denominators from e, combines the 8 cores, and divides.
    Softmax max-subtraction is skipped: scores are O(1) for this model, and
    softmax is shift-invariant, so exp() cannot overflow.
"""

import sys

import numpy as np

sys.path.insert(0, "/opt/trn_rl_repo")

import ml_dtypes

N_CORES = 8
D = 128  # feature dim
H = 128  # hidden dim
NSEG = 1024
PAD_SEG = NSEG  # extra segment id for padding rows
CHUNK = 128  # rows per PE contraction
CH = 4  # chunks per macro
MACRO = CHUNK * CH  # 512 rows
KST = 32  # macros per group (DMA/activation batch)

_prog_cache: dict = {}


def _build_program(NM: int, W: int, act_name: str = "Gelu"):
    """Emit + compile the per-core Tile program. NM macros per core (multiple
    of KST), segment window W."""
    from contextlib import ExitStack

    import concourse.tile as tile
    from concourse import bacc, mybir

    bf16 = mybir.dt.bfloat16
    f32 = mybir.dt.float32
    AF = mybir.ActivationFunctionType
    ALU = mybir.AluOpType

    assert NM % KST == 0
    NG = NM // KST
    Nc = NM * MACRO
    GROWS = KST * MACRO  # rows per group

    nc = bacc.Bacc("TRN2", target_bir_lowering=False, debug=False, num_devices=N_CORES)

    xap = nc.dram_tensor("xap", [CHUNK, NM, CH, D], bf16, kind="ExternalInput")
    xhit = nc.dram_tensor("xhit", [D, Nc], bf16, kind="ExternalInput")
    brel = nc.dram_tensor("brel", [128, NM, CH], f32, kind="ExternalInput")
    w1 = nc.dram_tensor("w1", [D, H], bf16, kind="ExternalInput")
    w2 = nc.dram_tensor("w2", [H, 1], bf16, kind="ExternalInput")
    b1 = nc.dram_tensor("b1", [H, 1], f32, kind="ExternalInput")
    b2 = nc.dram_tensor("b2", [128, 1], f32, kind="ExternalInput")
    iota = nc.dram_tensor("iota", [128, W], f32, kind="ExternalInput")
    pool_out = nc.dram_tensor("pool_out", [D, NM, W], f32, kind="ExternalOutput")
    e_out = nc.dram_tensor("e_out", [128, NM, CH], bf16, kind="ExternalOutput")

    # DRAM views (both host-prepped layouts are contiguous per partition)
    xa_view = xap.ap().rearrange("p (g k) j d -> g p k j d", k=KST)
    xt_view = xhit.ap().rearrange("d (g k r) -> g d k r", k=KST, r=MACRO)

    with tile.TileContext(nc) as tc, ExitStack() as ctx:
        pool = lambda name, bufs, **kw: ctx.enter_context(
            tc.tile_pool(name=name, bufs=bufs, **kw)
        )
        p_const = pool("const", 1)
        p_xa = pool("xa", 3)
        p_xt = pool("xt", 3)
        p_bt = pool("bt", 3)
        p_hg = pool("hg", 3)
        p_msk = pool("msk", 2)
        p_a = pool("amat", 2)
        p_es = pool("estage", 2)
        p_ps = pool("pstage", 2)
        p_hp = pool("hpsum", 2, space="PSUM")
        p_sc = pool("scpsum", 2, space="PSUM")
        p_pp = pool("ppsum", 2, space="PSUM")

        w1_sb = p_const.tile([D, H], bf16)
        nc.sync.dma_start(w1_sb[:], w1.ap())
        w2_sb = p_const.tile([H, 1], bf16)
        nc.sync.dma_start(w2_sb[:], w2.ap())
        b1_sb = p_const.tile([H, 1], f32)
        nc.sync.dma_start(b1_sb[:], b1.ap())
        b2_sb = p_const.tile([128, 1], f32)
        nc.sync.dma_start(b2_sb[:], b2.ap())
        iota_sb = p_const.tile([128, W], f32)
        nc.sync.dma_start(iota_sb[:], iota.ap())

        # Software pipeline: iteration g runs the scores pass for group g
        # interleaved (macro-by-macro, so PE/ACT/DVE all stay busy) with the
        # pooling pass for group g-1.
        prev = None  # (xa, bt, estage, m0) of group g-1
        for g in range(NG + 1):
            if g < NG:
                m0 = g * KST
                xa = p_xa.tile([128, KST, CH, CHUNK], bf16)
                nc.sync.dma_start(xa[:], xa_view[g])
                xt = p_xt.tile([128, KST, MACRO], bf16)
                nc.gpsimd.dma_start(xt[:], xt_view[g])
                bt = p_bt.tile([128, KST, CH], f32)
                nc.gpsimd.dma_start(bt[:], brel.ap()[:, m0 : m0 + KST, :])
                sc_g = p_sc.tile([128, KST, CH], f32, space="PSUM")

            if prev is not None:
                pstage = p_ps.tile([D, KST, W], f32)

            def emit_amat(k):
                # A matrix for (g-1, k) on the vector engine
                _, pbt, pes, _ = prev
                msk = p_msk.tile([128, CH, W], bf16)
                nc.vector.tensor_tensor(
                    out=msk[:],
                    in0=iota_sb[:].unsqueeze(1).broadcast_to([128, CH, W]),
                    in1=pbt[:, k, :].unsqueeze(2).broadcast_to([128, CH, W]),
                    op=ALU.is_equal,
                )
                amat = p_a.tile([128, CH, W], bf16)
                nc.vector.tensor_tensor(
                    out=amat[:],
                    in0=msk[:],
                    in1=pes[:, k, :].unsqueeze(2).broadcast_to([128, CH, W]),
                    op=ALU.mult,
                )
                return amat

            def emit_mm2(hg, i, k):
                for j in range(CH):
                    nc.tensor.matmul(
                        sc_g[:, k, j : j + 1],
                        lhsT=hg[:, i, j * CHUNK : (j + 1) * CHUNK],
                        rhs=w2_sb[:],
                        start=True,
                        stop=True,
                    )

            def emit_pool(k):
                amat = emit_amat(k)
                pxa = prev[0]
                pp = p_pp.tile([D, W], f32, space="PSUM")
                for j in range(CH):
                    nc.tensor.matmul(
                        pp[:], lhsT=pxa[:, k, j, :], rhs=amat[:, j, :],
                        start=(j == 0), stop=(j == CH - 1),
                    )
                nc.vector.tensor_copy(pstage[:, k, :], pp[:])

            # Macro pairs: mm1(k), mm1(k+1) into a 2-bank psum tile, one gelu
            # over both; pooling matmuls of (g-1) interleave to cover the
            # gelu latency in PE program order, and each pair's mm2s are
            # delayed one pair so they never wait on their gelu.
            pend_mm2 = None
            for kk in range(0, KST, 2):
                if g < NG:
                    hp = p_hp.tile([128, 2, MACRO], f32, space="PSUM")
                    nc.tensor.matmul(
                        hp[:, 0, :], lhsT=w1_sb[:], rhs=xt[:, kk, :],
                        start=True, stop=True,
                    )
                if prev is not None:
                    emit_pool(kk)
                if g < NG:
                    nc.tensor.matmul(
                        hp[:, 1, :], lhsT=w1_sb[:], rhs=xt[:, kk + 1, :],
                        start=True, stop=True,
                    )
                    hg = p_hg.tile([128, 2, MACRO], bf16)
                    nc.scalar.activation(
                        hg[:].rearrange("p i r -> p (i r)"),
                        hp[:].rearrange("p i r -> p (i r)"),
                        getattr(AF, act_name),
                        bias=b1_sb[:],
                        scale=1.0,
                    )
                if pend_mm2 is not None:
                    emit_mm2(pend_mm2, 0, kk - 2)
                    emit_mm2(pend_mm2, 1, kk - 1)
                if prev is not None:
                    emit_pool(kk + 1)
                if g < NG:
                    pend_mm2 = hg
            if pend_mm2 is not None:
                emit_mm2(pend_mm2, 0, KST - 2)
                emit_mm2(pend_mm2, 1, KST - 1)

            if prev is not None:
                # flush group g-1 on the (otherwise idle) gpsimd SWDGE queue
                pm0 = prev[3]
                nc.gpsimd.dma_start(pool_out.ap()[:, pm0 : pm0 + KST, :], pstage[:])

            if g < NG:
                # one exp per group: e = exp(scores + b2)
                estage = p_es.tile([128, KST, CH], bf16)
                nc.scalar.activation(
                    estage[:].rearrange("p k j -> p (k j)"),
                    sc_g[:].rearrange("p k j -> p (k j)"),
                    AF.Exp,
                    bias=b2_sb[:],
                    scale=1.0,
                )
                nc.gpsimd.dma_start(e_out.ap()[:, m0 : m0 + KST, :], estage[:])
                prev = (xa, bt, estage, m0)

    nc.compile()
    return nc


def _prep_inputs(x, batch, W1, b1, W2, b2):
    """Host-side shard + preprocess. Returns (in_maps, meta)."""
    bf = ml_dtypes.bfloat16
    x = np.asarray(x)
    batch = np.asarray(batch)
    N = x.shape[0]

    NM = -(-N // (N_CORES * MACRO))  # macros per core
    NM = -(-NM // KST) * KST  # round up to full groups
    NP = N_CORES * NM * MACRO
    Nc = NM * MACRO

    xhi = np.zeros((NP, D), dtype=bf)
    xhi[:N] = x.astype(bf)
    bpad = np.full(NP, PAD_SEG, dtype=np.int64)
    bpad[:N] = batch

    bv = bpad.reshape(N_CORES, NM, MACRO)
    # window start per macro; pad id is the largest so min() tracks real rows
    blo = bv.min(axis=2)  # [8, NM]
    # window width from real rows only
    real = bv != PAD_SEG
    breal_max = np.where(real, bv, -1).max(axis=2)  # -1 if all pad
    span = np.maximum(breal_max - blo + 1, 1)
    W = int(max(8, span.max()))
    assert W <= 128, f"segment window {W} too wide"

    brel = (bv - blo[:, :, None]).astype(np.float32)  # [8, NM, 512]
    # device layout: brel_dev[c, p, m, j] = brel[c, m, j*128 + p]
    brel_dev = np.ascontiguousarray(
        brel.reshape(N_CORES, NM, CH, CHUNK).transpose(0, 3, 1, 2)
    )

    iota_arr = np.ascontiguousarray(
        np.broadcast_to(np.arange(W, dtype=np.float32), (128, W))
    )
    w1c = np.ascontiguousarray(np.asarray(W1).astype(bf))
    w2c = np.ascontiguousarray(np.asarray(W2).astype(bf))
    b1c = np.ascontiguousarray(np.asarray(b1, dtype=np.float32).reshape(H, 1))
    b2c = np.full((128, 1), np.asarray(b2, dtype=np.float32).ravel()[0], np.float32)

    in_maps = []
    for c in range(N_CORES):
        xc = xhi[c * Nc : (c + 1) * Nc]
        in_maps.append(
            {
                # xap[p, m, j, :] = x[m*512 + j*128 + p, :]
                "xap": np.ascontiguousarray(
                    xc.reshape(NM, CH, CHUNK, D).transpose(2, 0, 1, 3)
                ),
                "xhit": np.ascontiguousarray(xc.T),
                "brel": brel_dev[c],
                "w1": w1c,
                "w2": w2c,
                "b1": b1c,
                "b2": b2c,
                "iota": iota_arr,
            }
        )
    meta = {"NM": NM, "W": W, "Nc": Nc, "NP": NP, "N": N, "blo": blo, "bpad": bpad}
    return in_maps, meta


def _combine(results, meta):
    """Host unshard: scatter-add macro windows, divide by segment denominators."""
    NM, W, Nc = meta["NM"], meta["W"], meta["Nc"]
    blo, bpad = meta["blo"], meta["bpad"]

    seg_acc = np.zeros((NSEG + 1, D), dtype=np.float64)
    e_all = np.empty(N_CORES * Nc, dtype=np.float32)
    wofs = np.arange(W)
    for c in range(N_CORES):
        po = np.asarray(results[c]["pool_out"], dtype=np.float64)  # [D, NM, W]
        seg_idx = (blo[c][:, None] + wofs[None, :]).ravel()  # [NM*W]
        valid = seg_idx <= NSEG
        contrib = po.transpose(1, 2, 0).reshape(-1, D)  # [NM*W, D]
        np.add.at(seg_acc, seg_idx[valid], contrib[valid])
        # e_dev[p, m, j] -> row m*512 + j*128 + p
        e_dev = np.asarray(results[c]["e_out"]).astype(np.float32)  # [128, NM, CH]
        e_all[c * Nc : (c + 1) * Nc] = e_dev.transpose(1, 2, 0).reshape(Nc)

    denom = np.bincount(bpad, weights=e_all.astype(np.float64), minlength=NSEG + 1)
    denom = denom[:NSEG]
    out = seg_acc[:NSEG]
    safe = denom != 0
    pooled = np.zeros((NSEG, D), dtype=np.float32)
    pooled[safe] = (out[safe] / denom[safe, None]).astype(np.float32)
    return pooled


def _run(inputs: dict, trace: bool = False):
    from concourse.bass_utils import run_bass_kernel_spmd

    in_maps, meta = _prep_inputs(
        inputs["x"], inputs["batch"], inputs["W1"], inputs["b1"], inputs["W2"],
        inputs["b2"],
    )
    key = (meta["NM"], meta["W"])
    if key not in _prog_cache:
        _prog_cache[key] = _build_program(*key)
    nc = _prog_cache[key]
    res = run_bass_kernel_spmd(
        nc, in_maps, core_ids=list(range(N_CORES)), trace=trace
    )
    pooled = _combine(res.results, meta)
    return pooled, res


def kernel(**inputs) -> np.ndarray:
    pooled, _ = _run(inputs, trace=False)
    return pooled
